# revision 16
# baseline (speedup 1.0000x reference)
"""UR-LSTM forward kernel for Trainium2 (8 NeuronCores).

Strategy (sequence-parallel with warmup):
  The UR-LSTM state is strongly contractive (~0.72x error decay/step), so a
  chunk of the sequence can be computed to tolerance by starting W steps
  earlier from zero state.  T=1024 is split into 16 chunks of C=64; each of
  the 8 cores runs 2 independent chains of S = C + W = 76 steps.  Chain j
  starts at max(0, 64j - W); chain 0 discards its last W steps instead of
  leading warmup, so the program is uniform across cores (SPMD).

  Per step, per chain (B=128 full batch on every core):
    gates[2048, 128] = W_hh.T-contraction (4 K=128 matmuls per gate tile)
      + x/bias contribution as K=32 matmuls row-packed 4-per-PE-pass via
      tile_position (the [x_t; 1] vector is replicated in all four 32-row
      groups of the x buffer).  PSUM holds (f+fb, r-fb, u, o) directly.
    Elementwise: ScalarE sigmoid/tanh (bf16 out), VectorE bf16 g-polynomial
      (2x DVE mode), GpSimd fp32 cell-state update.
    y_t = W_out @ h_t as 4 tiny matmuls; b_out is added on host.

  Two chains per core pipeline: while the PE runs chain B's matmuls, the
  other engines run chain A's elementwise chain.
"""

import os

import numpy as np
import ml_dtypes

EWBF16 = int(os.environ.get("KEWBF16", "1"))  # bf16 elementwise chain

B, T, I, H = 128, 1024, 10, 512
G4 = 4 * H  # 2048
NCORES = 8
NCHUNK = 16
W_WARM = 12
C_OUT = T // NCHUNK  # 64
S_STEPS = C_OUT + W_WARM  # 76
KCH = 4  # h-chunks of 128 (x/bias handled by packed K=32 matmuls)
GT = 16  # gate tiles of 128

_cache = {}


def _build_nc(S):
    import concourse.bacc as bacc
    import concourse.mybir as mybir
    import concourse.tile as tile

    dt = mybir.dt
    f32, bf16 = dt.float32, dt.bfloat16
    AF = mybir.ActivationFunctionType
    OP = mybir.AluOpType

    nc = bacc.Bacc(None, target_bir_lowering=False)

    w_d = nc.dram_tensor("w", [128, KCH * GT * 128], bf16, kind="ExternalInput")
    wx_d = nc.dram_tensor("wx", [128, GT * 128], bf16, kind="ExternalInput")
    wy_d = nc.dram_tensor("wy", [128, KCH * 10], bf16, kind="ExternalInput")
    x_d = [
        nc.dram_tensor(f"x{c}", [128, S * 128], bf16, kind="ExternalInput")
        for c in range(2)
    ]
    y_d = [
        nc.dram_tensor(f"y{c}", [S, 10, 128], f32, kind="ExternalOutput")
        for c in range(2)
    ]

    with tile.TileContext(nc) as tc:
        with (
            tc.tile_pool(name="const", bufs=1) as const,
            tc.tile_pool(name="hpool", bufs=2) as hpool,
            tc.tile_pool(name="ew", bufs=3) as ew,
            tc.tile_pool(name="gpsum", bufs=6, space="PSUM") as gpsum,
            tc.tile_pool(name="ypsum", bufs=2, space="PSUM") as ypsum,
            tc.tile_pool(name="yout", bufs=4) as youtp,
        ):
            wbuf = const.tile([128, KCH * GT * 128], bf16, tag="wbuf")
            nc.sync.dma_start(wbuf[:], w_d[:])
            wxbuf = const.tile([128, GT * 128], bf16, tag="wxbuf")
            nc.sync.dma_start(wxbuf[:], wx_d[:])
            wybuf = const.tile([128, KCH * 10], bf16, tag="wybuf")
            nc.sync.dma_start(wybuf[:], wy_d[:])
            xb = []
            for c in range(2):
                t = const.tile([128, S * 128], bf16, tag=f"xb{c}")
                nc.sync.dma_start(t[:], x_d[c][:])
                xb.append(t)

            cbuf = []
            h_prev = []
            for c in range(2):
                ct = const.tile([128, H], f32, tag=f"cbuf{c}")
                nc.vector.memset(ct[:], 0.0)
                cbuf.append(ct)
                ht = hpool.tile([128, H], bf16, tag=f"h{c}")
                nc.vector.memset(ht[:], 0.0)
                h_prev.append(ht)

            ewdt = bf16 if EWBF16 else f32
            yp_pend = [None, None]  # yp PSUM tile awaiting copy, per chain
            og_t = [None, None]

            def emit_y_mms(c, h_tile):
                yp = ypsum.tile([10, 128], f32, tag="yp")
                for k in range(KCH):
                    nc.tensor.matmul(
                        yp[:],
                        lhsT=wybuf[:, k * 10 : (k + 1) * 10],
                        rhs=h_tile[:, k * 128 : (k + 1) * 128],
                        start=(k == 0),
                        stop=(k == KCH - 1),
                    )
                return yp

            def emit_y_copy(c, s_idx, yp):
                yo = youtp.tile([10, 128], f32, tag="yo")
                nc.scalar.activation(yo[:], yp[:], AF.Copy)
                nc.sync.dma_start(y_d[c][s_idx], yo[:])

            for s in range(S):
                # ---- phase 1: gates matmuls + front elementwise, both chains
                for c in range(2):
                    banks = [
                        gpsum.tile([128, 512], f32, tag="gbank", name=f"gbank{i}")
                        for i in range(4)
                    ]
                    # One accumulation group per bank: the 4 K=128 x/bias
                    # matmuls open it (first clears the bank), then 16
                    # h-matmuls accumulate, the last closes it.
                    for b4 in range(4):
                        bank = banks[b4]
                        for j in range(4):
                            gt = 4 * b4 + j
                            nc.tensor.matmul(
                                bank[:, j * 128 : (j + 1) * 128],
                                lhsT=wxbuf[:, gt * 128 : (gt + 1) * 128],
                                rhs=xb[c][:, s * 128 : (s + 1) * 128],
                                start=(j == 0),
                                stop=False,
                            )
                        for j in range(4):
                            gt = 4 * b4 + j
                            for k in range(KCH):
                                nc.tensor.matmul(
                                    bank[:, j * 128 : (j + 1) * 128],
                                    lhsT=wbuf[
                                        :, (k * GT + gt) * 128 : (k * GT + gt + 1) * 128
                                    ],
                                    rhs=h_prev[c][:, k * 128 : (k + 1) * 128],
                                    start=False,
                                    stop=(j == 3 and k == KCH - 1),
                                )

                    fg = ew.tile([128, 512], ewdt, tag="fg")
                    rg = ew.tile([128, 512], ewdt, tag="rg")
                    tu = ew.tile([128, 512], ewdt, tag="tu")
                    og = ew.tile([128, 512], ewdt, tag="og")
                    nc.scalar.activation(fg[:], banks[0][:], AF.Sigmoid)
                    nc.scalar.activation(rg[:], banks[1][:], AF.Sigmoid)
                    nc.scalar.activation(tu[:], banks[2][:], AF.Tanh)
                    nc.scalar.activation(og[:], banks[3][:], AF.Sigmoid)
                    og_t[c] = og

                    # g = fg^2 + 2*rg*fg*(1-fg) = fg*(fg - 2a) + 2a,  a = fg*rg
                    av = ew.tile([128, 512], ewdt, tag="av")
                    bv = ew.tile([128, 512], ewdt, tag="bv")
                    cv = ew.tile([128, 512], ewdt, tag="cv")
                    gv = ew.tile([128, 512], ewdt, tag="gv")
                    nc.vector.tensor_tensor(av[:], fg[:], rg[:], OP.mult)
                    nc.vector.scalar_tensor_tensor(
                        bv[:], av[:], -2.0, fg[:], OP.mult, OP.add
                    )
                    nc.vector.tensor_tensor(cv[:], fg[:], bv[:], OP.mult)
                    nc.vector.scalar_tensor_tensor(
                        gv[:], av[:], 2.0, cv[:], OP.mult, OP.add
                    )

                    # cell state update on GpSimd (fp32 state)
                    wv = ew.tile([128, 512], f32, tag="wv")
                    zv = ew.tile([128, 512], f32, tag="zv")
                    nc.gpsimd.tensor_tensor(wv[:], cbuf[c][:], tu[:], OP.subtract)
                    nc.gpsimd.tensor_tensor(zv[:], gv[:], wv[:], OP.mult)
                    nc.gpsimd.tensor_tensor(cbuf[c][:], zv[:], tu[:], OP.add)

                # y PSUM->SBUF copies from two steps ago land in the ACT idle
                # window between the sigmoids and tanh(c)
                if s >= 2:
                    for c in range(2):
                        emit_y_copy(c, s - 2, yp_pend[c])
                        yp_pend[c] = None

                # ---- phase 2: state tail + h + deferred y matmuls
                for c in range(2):
                    tc2 = ew.tile([128, 512], ewdt, tag="tc2")
                    nc.scalar.activation(tc2[:], cbuf[c][:], AF.Tanh)
                    h_new = hpool.tile([128, H], bf16, tag=f"h{c}")
                    nc.vector.tensor_tensor(h_new[:], og_t[c][:], tc2[:], OP.mult)

                    if s >= 1:
                        yp_pend[c] = emit_y_mms(c, h_prev[c])
                    h_prev[c] = h_new

            # flush: y for steps S-2 and S-1
            for c in range(2):
                emit_y_copy(c, S - 2, yp_pend[c])
            for c in range(2):
                yp = emit_y_mms(c, h_prev[c])
                emit_y_copy(c, S - 1, yp)

    nc.compile()
    return nc


def _prep(inputs):
    x = np.asarray(inputs["x"], np.float32)
    W_ih = np.asarray(inputs["W_ih"], np.float32)
    W_hh = np.asarray(inputs["W_hh"], np.float32)
    b = np.asarray(inputs["b"], np.float32)
    fb = np.asarray(inputs["fb"], np.float32)
    W_out = np.asarray(inputs["W_out"], np.float32)
    bf = ml_dtypes.bfloat16

    bias_col = b.copy()
    bias_col[0:H] += fb
    bias_col[H : 2 * H] -= fb

    # h-contraction weights: w[p, (k*GT+gt)*128+m] = W_hh.T[k*128+p, gt*128+m]
    w_host = (
        W_hh.T.reshape(KCH, 128, GT, 128).transpose(1, 0, 2, 3).reshape(128, -1)
    ).astype(bf)

    # x/bias weights, zero-padded to K=128
    Wx = np.zeros((128, G4), np.float32)
    Wx[0:I] = W_ih.T
    Wx[I] = bias_col
    wx_host = Wx.astype(bf)  # [128, 2048]

    # y projection weights (h-contraction only; b_out added on host)
    wy_host = (
        W_out.T.reshape(KCH, 128, 10).transpose(1, 0, 2).reshape(128, -1)
    ).astype(bf)

    # per-chain x buffers: [x_t(10); 1; 0-pad] per step column block
    xc = []
    for j in range(NCHUNK):
        start = max(0, j * C_OUT - W_WARM)
        xs = x[:, start : start + S_STEPS, :]  # [128, S, 10]
        arr = np.zeros((128, S_STEPS, 128), np.float32)
        arr[0:I] = xs.transpose(2, 1, 0)
        arr[I] = 1.0
        xc.append(arr.reshape(128, -1).astype(bf))
    return w_host, wx_host, wy_host, xc


def _in_maps(inputs):
    w_host, wx_host, wy_host, xc = _prep(inputs)
    in_maps = []
    for core in range(NCORES):
        in_maps.append(
            {
                "w": w_host,
                "wx": wx_host,
                "wy": wy_host,
                "x0": xc[2 * core],
                "x1": xc[2 * core + 1],
            }
        )
    return in_maps


def kernel(**inputs):
    from concourse.bass_utils import run_bass_kernel_spmd

    if "nc" not in _cache:
        _cache["nc"] = _build_nc(S_STEPS)
    nc = _cache["nc"]

    in_maps = _in_maps(inputs)
    res = run_bass_kernel_spmd(nc, in_maps, list(range(NCORES))).results

    b_out = np.asarray(inputs["b_out"], np.float32)
    y = np.zeros((B, T, 10), np.float32)
    for j in range(NCHUNK):
        core, chain = j // 2, j % 2
        yj = np.asarray(res[core][f"y{chain}"], np.float32)  # [S, 10, 128]
        yj = yj.transpose(2, 0, 1)  # [B, S, 10]
        w0 = 0 if j == 0 else W_WARM
        y[:, j * C_OUT : (j + 1) * C_OUT, :] = yj[:, w0 : w0 + C_OUT, :]
    return y + b_out


# revision 19
# speedup vs baseline: 1.4557x; 1.4557x over previous
"""UR-LSTM forward kernel for Trainium2 (8 NeuronCores).

Strategy (sequence-parallel with warmup):
  The UR-LSTM state is strongly contractive (~0.72x error decay/step), so a
  chunk of the sequence can be computed to tolerance by starting W steps
  earlier from zero state.  T=1024 is split into 16 chunks of C=64; each of
  the 8 cores runs 2 independent chains of S = C + W = 76 steps.  Chain j
  starts at max(0, 64j - W); chain 0 discards its last W steps instead of
  leading warmup, so the program is uniform across cores (SPMD).

  Per step, per chain (B=128 full batch on every core):
    gates[2048, 128] = W_hh.T-contraction (4 K=128 matmuls per gate tile)
      + x/bias contribution as K=32 matmuls row-packed 4-per-PE-pass via
      tile_position (the [x_t; 1] vector is replicated in all four 32-row
      groups of the x buffer).  PSUM holds (f+fb, r-fb, u, o) directly.
    Elementwise: ScalarE sigmoid/tanh (bf16 out), VectorE bf16 g-polynomial
      (2x DVE mode), GpSimd fp32 cell-state update.
    y_t = W_out @ h_t as 4 tiny matmuls; b_out is added on host.

  Two chains per core pipeline: while the PE runs chain B's matmuls, the
  other engines run chain A's elementwise chain.
"""

import os

import numpy as np
import ml_dtypes

EWBF16 = int(os.environ.get("KEWBF16", "1"))  # bf16 elementwise chain

B, T, I, H = 128, 1024, 10, 512
G4 = 4 * H  # 2048
NCORES = 8
NCHUNK = 16
W_WARM = 10
C_OUT = T // NCHUNK  # 64
S_STEPS = C_OUT + W_WARM  # 76
KCH = 4  # h-chunks of 128 (x/bias handled by packed K=32 matmuls)
GT = 16  # gate tiles of 128

_cache = {}


def _build_nc(S):
    import concourse.bacc as bacc
    import concourse.mybir as mybir
    import concourse.tile as tile

    dt = mybir.dt
    f32, bf16 = dt.float32, dt.bfloat16
    AF = mybir.ActivationFunctionType
    OP = mybir.AluOpType

    nc = bacc.Bacc(None, target_bir_lowering=False)

    w_d = nc.dram_tensor("w", [128, KCH * GT * 128], bf16, kind="ExternalInput")
    wx_d = nc.dram_tensor("wx", [128, GT * 128], bf16, kind="ExternalInput")
    wy_d = nc.dram_tensor("wy", [128, KCH * 10], bf16, kind="ExternalInput")
    x_d = [
        nc.dram_tensor(f"x{c}", [128, S * 128], bf16, kind="ExternalInput")
        for c in range(2)
    ]
    y_d = [
        nc.dram_tensor(f"y{c}", [S, 10, 128], f32, kind="ExternalOutput")
        for c in range(2)
    ]

    with tile.TileContext(nc) as tc:
        with (
            tc.tile_pool(name="const", bufs=1) as const,
            tc.tile_pool(name="hpool", bufs=2) as hpool,
            tc.tile_pool(name="ew", bufs=3) as ew,
            tc.tile_pool(name="gpsum", bufs=6, space="PSUM") as gpsum,
            tc.tile_pool(name="ypsum", bufs=2, space="PSUM") as ypsum,
            tc.tile_pool(name="yout", bufs=4) as youtp,
        ):
            wbuf = const.tile([128, KCH * GT * 128], bf16, tag="wbuf")
            nc.sync.dma_start(wbuf[:], w_d[:])
            wxbuf = const.tile([128, GT * 128], bf16, tag="wxbuf")
            nc.sync.dma_start(wxbuf[:], wx_d[:])
            wybuf = const.tile([128, KCH * 10], bf16, tag="wybuf")
            nc.sync.dma_start(wybuf[:], wy_d[:])
            xb = []
            for c in range(2):
                t = const.tile([128, S * 128], bf16, tag=f"xb{c}")
                nc.sync.dma_start(t[:], x_d[c][:])
                xb.append(t)

            cdt = bf16 if EWBF16 else f32
            cbuf = []
            h_prev = []
            for c in range(2):
                ct = const.tile([128, H], cdt, tag=f"cbuf{c}")
                nc.vector.memset(ct[:], 0.0)
                cbuf.append(ct)
                ht = hpool.tile([128, H], bf16, tag=f"h{c}")
                nc.vector.memset(ht[:], 0.0)
                h_prev.append(ht)

            ewdt = bf16 if EWBF16 else f32
            yp_pend = [None, None]  # yp PSUM tile awaiting copy, per chain
            og_t = [None, None]

            def emit_y_mms(c, h_tile):
                yp = ypsum.tile([10, 128], f32, tag="yp")
                for k in range(KCH):
                    nc.tensor.matmul(
                        yp[:],
                        lhsT=wybuf[:, k * 10 : (k + 1) * 10],
                        rhs=h_tile[:, k * 128 : (k + 1) * 128],
                        start=(k == 0),
                        stop=(k == KCH - 1),
                    )
                return yp

            def emit_y_copy(c, s_idx, yp):
                yo = youtp.tile([10, 128], f32, tag="yo")
                nc.scalar.activation(yo[:], yp[:], AF.Copy)
                nc.sync.dma_start(y_d[c][s_idx], yo[:])

            for s in range(S):
                # ---- phase 1: gates matmuls + front elementwise, both chains
                for c in range(2):
                    banks = [
                        gpsum.tile([128, 512], f32, tag="gbank", name=f"gbank{i}")
                        for i in range(4)
                    ]
                    # One accumulation group per bank: the 4 K=128 x/bias
                    # matmuls open it (first clears the bank), then 16
                    # h-matmuls accumulate, the last closes it.
                    for b4 in range(4):
                        bank = banks[b4]
                        for j in range(4):
                            gt = 4 * b4 + j
                            nc.tensor.matmul(
                                bank[:, j * 128 : (j + 1) * 128],
                                lhsT=wxbuf[:, gt * 128 : (gt + 1) * 128],
                                rhs=xb[c][:, s * 128 : (s + 1) * 128],
                                start=(j == 0),
                                stop=False,
                            )
                        for j in range(4):
                            gt = 4 * b4 + j
                            for k in range(KCH):
                                nc.tensor.matmul(
                                    bank[:, j * 128 : (j + 1) * 128],
                                    lhsT=wbuf[
                                        :, (k * GT + gt) * 128 : (k * GT + gt + 1) * 128
                                    ],
                                    rhs=h_prev[c][:, k * 128 : (k + 1) * 128],
                                    start=False,
                                    stop=(j == 3 and k == KCH - 1),
                                )

                    fg = ew.tile([128, 512], ewdt, tag="fg")
                    rg = ew.tile([128, 512], ewdt, tag="rg")
                    tu = ew.tile([128, 512], ewdt, tag="tu")
                    og = ew.tile([128, 512], ewdt, tag="og")
                    nc.scalar.activation(fg[:], banks[0][:], AF.Sigmoid)
                    nc.scalar.activation(rg[:], banks[1][:], AF.Sigmoid)
                    nc.scalar.activation(tu[:], banks[2][:], AF.Tanh)
                    nc.scalar.activation(og[:], banks[3][:], AF.Sigmoid)
                    og_t[c] = og

                    # All elementwise on DVE in bf16 (2x mode, no GpSimd —
                    # avoids the DVE/GpSimd shared-SBUF-port contention).
                    # g = fg^2 + 2*rg*fg*(1-fg) = fg*(fg - 2a) + 2a,  a = fg*rg
                    av = ew.tile([128, 512], ewdt, tag="av")
                    bv = ew.tile([128, 512], ewdt, tag="bv")
                    cv = ew.tile([128, 512], ewdt, tag="cv")
                    gv = ew.tile([128, 512], ewdt, tag="gv")
                    wv = ew.tile([128, 512], ewdt, tag="wv")
                    zv = ew.tile([128, 512], ewdt, tag="zv")
                    nc.vector.tensor_tensor(wv[:], cbuf[c][:], tu[:], OP.subtract)
                    nc.vector.tensor_tensor(av[:], fg[:], rg[:], OP.mult)
                    nc.vector.scalar_tensor_tensor(
                        bv[:], av[:], -2.0, fg[:], OP.mult, OP.add
                    )
                    nc.vector.tensor_tensor(cv[:], fg[:], bv[:], OP.mult)
                    nc.vector.scalar_tensor_tensor(
                        gv[:], av[:], 2.0, cv[:], OP.mult, OP.add
                    )
                    nc.vector.tensor_tensor(zv[:], gv[:], wv[:], OP.mult)
                    nc.vector.tensor_tensor(cbuf[c][:], zv[:], tu[:], OP.add)

                # y PSUM->SBUF copies from two steps ago land in the ACT idle
                # window between the sigmoids and tanh(c)
                if s >= 2:
                    for c in range(2):
                        emit_y_copy(c, s - 2, yp_pend[c])
                        yp_pend[c] = None

                # ---- phase 2: state tail + h + deferred y matmuls
                for c in range(2):
                    tc2 = ew.tile([128, 512], ewdt, tag="tc2")
                    nc.scalar.activation(tc2[:], cbuf[c][:], AF.Tanh)
                    h_new = hpool.tile([128, H], bf16, tag=f"h{c}")
                    nc.vector.tensor_tensor(h_new[:], og_t[c][:], tc2[:], OP.mult)

                    if s >= 1:
                        yp_pend[c] = emit_y_mms(c, h_prev[c])
                    h_prev[c] = h_new

            # flush: y for steps S-2 and S-1
            for c in range(2):
                emit_y_copy(c, S - 2, yp_pend[c])
            for c in range(2):
                yp = emit_y_mms(c, h_prev[c])
                emit_y_copy(c, S - 1, yp)

    nc.compile()
    return nc


def _prep(inputs):
    x = np.asarray(inputs["x"], np.float32)
    W_ih = np.asarray(inputs["W_ih"], np.float32)
    W_hh = np.asarray(inputs["W_hh"], np.float32)
    b = np.asarray(inputs["b"], np.float32)
    fb = np.asarray(inputs["fb"], np.float32)
    W_out = np.asarray(inputs["W_out"], np.float32)
    bf = ml_dtypes.bfloat16

    bias_col = b.copy()
    bias_col[0:H] += fb
    bias_col[H : 2 * H] -= fb

    # h-contraction weights: w[p, (k*GT+gt)*128+m] = W_hh.T[k*128+p, gt*128+m]
    w_host = (
        W_hh.T.reshape(KCH, 128, GT, 128).transpose(1, 0, 2, 3).reshape(128, -1)
    ).astype(bf)

    # x/bias weights, zero-padded to K=128
    Wx = np.zeros((128, G4), np.float32)
    Wx[0:I] = W_ih.T
    Wx[I] = bias_col
    wx_host = Wx.astype(bf)  # [128, 2048]

    # y projection weights (h-contraction only; b_out added on host)
    wy_host = (
        W_out.T.reshape(KCH, 128, 10).transpose(1, 0, 2).reshape(128, -1)
    ).astype(bf)

    # per-chain x buffers: [x_t(10); 1; 0-pad] per step column block
    xc = []
    for j in range(NCHUNK):
        start = max(0, j * C_OUT - W_WARM)
        xs = x[:, start : start + S_STEPS, :]  # [128, S, 10]
        arr = np.zeros((128, S_STEPS, 128), np.float32)
        arr[0:I] = xs.transpose(2, 1, 0)
        arr[I] = 1.0
        xc.append(arr.reshape(128, -1).astype(bf))
    return w_host, wx_host, wy_host, xc


def _in_maps(inputs):
    w_host, wx_host, wy_host, xc = _prep(inputs)
    in_maps = []
    for core in range(NCORES):
        in_maps.append(
            {
                "w": w_host,
                "wx": wx_host,
                "wy": wy_host,
                "x0": xc[2 * core],
                "x1": xc[2 * core + 1],
            }
        )
    return in_maps


def kernel(**inputs):
    from concourse.bass_utils import run_bass_kernel_spmd

    if "nc" not in _cache:
        _cache["nc"] = _build_nc(S_STEPS)
    nc = _cache["nc"]

    in_maps = _in_maps(inputs)
    res = run_bass_kernel_spmd(nc, in_maps, list(range(NCORES))).results

    b_out = np.asarray(inputs["b_out"], np.float32)
    y = np.zeros((B, T, 10), np.float32)
    for j in range(NCHUNK):
        core, chain = j // 2, j % 2
        yj = np.asarray(res[core][f"y{chain}"], np.float32)  # [S, 10, 128]
        yj = yj.transpose(2, 0, 1)  # [B, S, 10]
        w0 = 0 if j == 0 else W_WARM
        y[:, j * C_OUT : (j + 1) * C_OUT, :] = yj[:, w0 : w0 + C_OUT, :]
    return y + b_out
